# revision 11
# baseline (speedup 1.0000x reference)
"""De-stationary attention (B=4, L=S=2048, D=512, H=8, dk=64) on 8 TRN2 cores.

Sharding: core c -> batch b = c//2, query-half = c%2 (1024 rows each).
Each core computes full attention for its (batch, q-half) over all 8 heads
using the whole K/V of that batch; outputs concatenate with no reduction.

Math (per batch):
  q = queries @ Wq + bq ; k = keys @ Wk + bk ; v = values @ Wv + bv
  scores = tau * (q . k) / 8 + delta[s]
  attn   = softmax_s(scores)            (no max-subtraction; |scores| <~ 10)
  out    = (attn @ v) @ Wo + bo

Device-side trick: exp(tau*qk/8 + delta_s) = exp(tau/8 * qk) * w_s with
w_s = exp(delta_s) folded into V: AV matmul uses lhsT = [w*v | w]
so row 64 of the (transposed) AV output is the softmax denominator.

Layouts are transposed end-to-end so no on-device transposes are needed:
  host supplies queries^T/keys^T/values^T; the final output is natural.
"""

import os
from contextlib import ExitStack

import numpy as np

import concourse.bass as bass
import concourse.bacc as bacc
import concourse.mybir as mybir
import concourse.tile as tile
from concourse.bass_utils import run_bass_kernel_spmd

# Problem constants (hardcoded per the harness contract).
B, LFULL, S, D = 4, 2048, 2048, 512
H, DK = 8, 64
NCORES = 8
LC = B * LFULL // NCORES  # 1024 query rows per core
NQT = LC // 512           # q-tiles of 512
SC = S // 128             # 16 s-chunks
F32 = mybir.dt.float32
F32R = mybir.dt.float32r

# Matmul dtype knob: "f32r" (full-rate, ~tf32 precision) or "f32" (4x slower).
MM_DTYPE = os.environ.get("KERNEL_MM_DTYPE", "f32r")
MDT = F32R if MM_DTYPE == "f32r" else F32
AF = mybir.ActivationFunctionType
OP = mybir.AluOpType


def _mm(nc, out, lhsT, rhs, **kw):
    nc.tensor.matmul(out, lhsT, rhs, **kw)


def build_nc():
    nc = bacc.Bacc()

    qTin = nc.dram_tensor("qTin", [D, LC], MDT, kind="ExternalInput")
    kTin = nc.dram_tensor("kTin", [D, S], MDT, kind="ExternalInput")
    vTin = nc.dram_tensor("vTin", [D, S], MDT, kind="ExternalInput")
    Wq = nc.dram_tensor("Wq", [D, D], MDT, kind="ExternalInput")
    Wk = nc.dram_tensor("Wk", [D, D], MDT, kind="ExternalInput")
    Wv = nc.dram_tensor("Wv", [D, D], MDT, kind="ExternalInput")
    Wo = nc.dram_tensor("Wo", [D, D], MDT, kind="ExternalInput")
    bq = nc.dram_tensor("bq", [D], F32, kind="ExternalInput")
    bk = nc.dram_tensor("bk", [D], F32, kind="ExternalInput")
    bv = nc.dram_tensor("bv", [D], MDT, kind="ExternalInput")
    bo = nc.dram_tensor("bo", [D], MDT, kind="ExternalInput")
    tau = nc.dram_tensor("tau", [1], F32, kind="ExternalInput")
    delta = nc.dram_tensor("delta", [S], F32, kind="ExternalInput")
    out = nc.dram_tensor("out", [LC, D], F32, kind="ExternalOutput")

    with ExitStack() as ctx:
        tc = ctx.enter_context(tile.TileContext(nc))
        consts = ctx.enter_context(tc.tile_pool(name="consts", bufs=1))
        proj = ctx.enter_context(tc.tile_pool(name="proj", bufs=1))

        # --- small constants -------------------------------------------------
        Wo_sb = consts.tile([64, H, D], MDT)  # Wo rows for head h at parts 0..63
        nc.sync.dma_start(out=Wo_sb, in_=Wo.rearrange("(h d) n -> d h n", d=64))
        bq_sb = consts.tile([128, 4], F32)
        nc.sync.dma_start(out=bq_sb, in_=bq.rearrange("(j p) -> p j", p=128))
        bk_sb = consts.tile([128, 4], F32)
        nc.sync.dma_start(out=bk_sb, in_=bk.rearrange("(j p) -> p j", p=128))
        bv_row = consts.tile([1, D], MDT)
        nc.sync.dma_start(out=bv_row, in_=bv.rearrange("(a n) -> a n", a=1))
        bo_row = consts.tile([1, D], MDT)
        nc.sync.dma_start(out=bo_row, in_=bo.rearrange("(a n) -> a n", a=1))
        tau_bc0 = consts.tile([128, 1], F32)
        nc.sync.dma_start(
            out=tau_bc0,
            in_=tau.rearrange("(a b) -> a b", a=1).to_broadcast([128, 1]))
        delta_sb = consts.tile([128, SC], F32)
        nc.sync.dma_start(out=delta_sb, in_=delta.rearrange("(j p) -> p j", p=128))
        ones_f32 = consts.tile([1, 128], F32)
        nc.vector.memset(ones_f32, 1.0)
        ones_sb = consts.tile([1, 128], MDT)
        nc.vector.tensor_copy(out=ones_sb, in_=ones_f32)

        tau_bc = consts.tile([128, 1], F32)
        nc.vector.tensor_scalar(out=tau_bc, in0=tau_bc0, scalar1=0.125,
                                scalar2=None, op0=OP.mult)  # tau/sqrt(dk)
        w_sb = consts.tile([128, SC], F32)  # w[s] = exp(delta[s])
        nc.scalar.activation(w_sb, delta_sb, AF.Exp)

        # --- persistent projection outputs ----------------------------------
        qT_sb = proj.tile([128, 4, LC], MDT)      # q^T[128*jo+p, l]
        kT_sb = proj.tile([128, 4, S], MDT)       # k^T[128*jo+p, s]
        vw_sb = proj.tile([128, SC, H, 65], MDT)  # [w*v | w] per s-chunk/head

        # --- phase 0: projections -------------------------------------------
        with (
            tc.tile_pool(name="w0", bufs=1) as w0,
            tc.tile_pool(name="pin", bufs=1) as pin,
            tc.tile_pool(name="pps", bufs=8, space="PSUM") as pps,
        ):
            Wv_sb = w0.tile([128, 4, D], MDT)
            nc.sync.dma_start(out=Wv_sb, in_=Wv.rearrange("(j p) n -> p j n", p=128))
            vT_sb = pin.tile([128, 4, S], MDT)
            nc.sync.dma_start(out=vT_sb, in_=vTin.rearrange("(j p) s -> p j s", p=128))
            Wq_sb = w0.tile([128, 4, D], MDT)
            nc.sync.dma_start(out=Wq_sb, in_=Wq.rearrange("(j p) n -> p j n", p=128))
            qTin_sb = pin.tile([128, 4, LC], MDT)
            nc.sync.dma_start(out=qTin_sb, in_=qTin.rearrange("(j p) l -> p j l", p=128))
            Wk_sb = w0.tile([128, 4, D], MDT)
            nc.sync.dma_start(out=Wk_sb, in_=Wk.rearrange("(j p) n -> p j n", p=128))
            kTin_sb = pin.tile([128, 4, S], MDT)
            nc.sync.dma_start(out=kTin_sb, in_=kTin.rearrange("(j p) s -> p j s", p=128))

            # v projection -> vw (natural layout [s, dm], weighted by w, +ones col)
            for st in range(SC):
                ps = pps.tile([128, 512], F32, name=f"psv_{st}", tag="pss")
                for ji in range(4):
                    _mm(nc, ps, vT_sb[:, ji, st * 128:(st + 1) * 128],
                        Wv_sb[:, ji, :], start=(ji == 0), stop=False)
                _mm(nc, ps, ones_sb, bv_row, start=False, stop=True)
                nc.vector.tensor_scalar(
                    out=vw_sb[:, st, :, 0:64],
                    in0=ps.rearrange("p (h d) -> p h d", h=H),
                    scalar1=w_sb[:, st:st + 1], scalar2=None, op0=OP.mult)
                nc.vector.tensor_copy(
                    out=vw_sb[:, st, :, 64:65],
                    in_=w_sb[:, st:st + 1].to_broadcast([128, H, 1]))

            # q projection (input-stationary over dm_in chunks)
            for lt in range(NQT):
                pss = [pps.tile([128, 512], F32, name=f"psq_{lt}_{j}", tag="pss")
                       for j in range(4)]
                for ji in range(4):
                    for jo in range(4):
                        _mm(nc, pss[jo], Wq_sb[:, ji, jo * 128:(jo + 1) * 128],
                            qTin_sb[:, ji, lt * 512:(lt + 1) * 512],
                            start=(ji == 0), stop=(ji == 3))
                for jo in range(4):
                    nc.vector.tensor_scalar(
                        out=qT_sb[:, jo, lt * 512:(lt + 1) * 512], in0=pss[jo],
                        scalar1=bq_sb[:, jo:jo + 1], scalar2=None, op0=OP.add)

            # k projection
            for st in range(4):
                pss = [pps.tile([128, 512], F32, name=f"psk_{st}_{j}", tag="pss")
                       for j in range(4)]
                for ji in range(4):
                    for jo in range(4):
                        _mm(nc, pss[jo], Wk_sb[:, ji, jo * 128:(jo + 1) * 128],
                            kTin_sb[:, ji, st * 512:(st + 1) * 512],
                            start=(ji == 0), stop=(ji == 3))
                for jo in range(4):
                    nc.vector.tensor_scalar(
                        out=kT_sb[:, jo, st * 512:(st + 1) * 512], in0=pss[jo],
                        scalar1=bk_sb[:, jo:jo + 1], scalar2=None, op0=OP.add)

        # --- phase 1: attention ----------------------------------------------
        with (
            tc.tile_pool(name="qkp", bufs=3, space="PSUM") as qkp,
            tc.tile_pool(name="avp", bufs=2, space="PSUM") as avp,
            tc.tile_pool(name="pp", bufs=6) as pp,
            tc.tile_pool(name="onp", bufs=2 * H) as onp,
            tc.tile_pool(name="rcp", bufs=3) as rcp,
            tc.tile_pool(name="dtp", bufs=4) as dtp,
            tc.tile_pool(name="epp", bufs=3) as epp,
            tc.tile_pool(name="fsp", bufs=3) as fsp,
            tc.tile_pool(name="drp", bufs=4, space="DRAM") as drp,
        ):
            for qt in range(NQT):
                outTn = {}
                for hp in range(H // 2):
                    h0, h1 = 2 * hp, 2 * hp + 1
                    av = [avp.tile([128, 512], F32, name=f"av_{qt}_{hp}_{j}", tag="avf")
                          for j in range(2)]
                    for scp in range(SC // 2):
                        qk0 = qkp.tile([128, 1024], F32, name=f"qk0_{qt}_{hp}_{scp}", tag="qk")
                        qk1 = qkp.tile([128, 1024], F32, name=f"qk1_{qt}_{hp}_{scp}", tag="qk")
                        for k2 in range(2):
                            sc = 2 * scp + k2
                            # heads of the pair live on partition halves of
                            # kT/qT chunk hp -> concurrent row-tiled matmuls
                            _mm(nc, qk0[:, k2 * 512:(k2 + 1) * 512],
                                kT_sb[0:64, hp, sc * 128:(sc + 1) * 128],
                                qT_sb[0:64, hp, qt * 512:(qt + 1) * 512],
                                start=True, stop=True)
                            _mm(nc, qk1[:, k2 * 512:(k2 + 1) * 512],
                                kT_sb[64:128, hp, sc * 128:(sc + 1) * 128],
                                qT_sb[64:128, hp, qt * 512:(qt + 1) * 512],
                                start=True, stop=True)
                        p0 = pp.tile([128, 1024], MDT, name=f"p0_{qt}_{hp}_{scp}", tag="p")
                        p1 = pp.tile([128, 1024], MDT, name=f"p1_{qt}_{hp}_{scp}", tag="p")
                        nc.scalar.activation(p0, qk0, AF.Exp, scale=tau_bc)
                        nc.scalar.activation(p1, qk1, AF.Exp, scale=tau_bc)
                        for k2 in range(2):
                            sc = 2 * scp + k2
                            _mm(nc, av[0][0:65, :], vw_sb[:, sc, h0, :],
                                p0[:, k2 * 512:(k2 + 1) * 512],
                                start=(sc == 0), stop=(sc == SC - 1))
                            _mm(nc, av[1][0:65, :], vw_sb[:, sc, h1, :],
                                p1[:, k2 * 512:(k2 + 1) * 512],
                                start=(sc == 0), stop=(sc == SC - 1))
                    for i2, h in ((0, h0), (1, h1)):
                        rt = rcp.tile([65, 512], F32)
                        nc.vector.tensor_copy(
                            out=rt[64:65, :], in_=av[i2][64:65, :])
                        d1 = drp.tile([512], F32, name=f"d1_{qt}_{hp}_{i2}",
                                      tag="d1")
                        nc.sync.dma_start(
                            out=d1.rearrange("(a n) -> a n", a=1),
                            in_=rt[64:65, :])
                        denT = dtp.tile([128, 4], F32, name=f"dT_{qt}_{hp}_{i2}",
                                        tag="dT")
                        nc.sync.dma_start(
                            out=denT, in_=d1.rearrange("(j p) -> p j", p=128))
                        recipT = dtp.tile([128, 4], F32,
                                          name=f"rT_{qt}_{hp}_{i2}", tag="rT")
                        nc.vector.reciprocal(recipT, denT)
                        d2 = drp.tile([512], F32, name=f"d2_{qt}_{hp}_{i2}",
                                      tag="d2")
                        nc.sync.dma_start(
                            out=d2.rearrange("(j p) -> p j", p=128), in_=recipT)
                        rb = epp.tile([128, 512], F32)
                        nc.sync.dma_start(
                            out=rb,
                            in_=d2.rearrange("(a n) -> a n", a=1)
                            .to_broadcast([128, 512]))
                        ot = onp.tile([64, 512], MDT)
                        nc.vector.tensor_mul(ot, av[i2][0:64, :], rb[0:64, :])
                        outTn[h] = ot

                # output projection for this q-tile
                for i in range(4):
                    fps = avp.tile([128, 512], F32, name=f"fps_{qt}_{i}", tag="avf")
                    for h in range(H):
                        _mm(nc, fps, outTn[h][:, i * 128:(i + 1) * 128],
                            Wo_sb[:, h, :], start=(h == 0), stop=False)
                    _mm(nc, fps, ones_sb, bo_row, start=False, stop=True)
                    fsb = fsp.tile([128, 512], F32)
                    nc.vector.tensor_copy(out=fsb, in_=fps)
                    r0 = qt * 512 + i * 128
                    nc.sync.dma_start(out=out[r0:r0 + 128, :], in_=fsb)

    return nc


_NC_CACHE = None


def _get_nc():
    global _NC_CACHE
    if _NC_CACHE is None:
        _NC_CACHE = build_nc()
        _NC_CACHE.finalize()
    return _NC_CACHE


def kernel(queries, keys, values, tau, delta, Wq, bq, Wk, bk, Wv, bv, Wo, bo,
           **_unused):
    queries = np.ascontiguousarray(np.asarray(queries, np.float32))
    keys = np.ascontiguousarray(np.asarray(keys, np.float32))
    values = np.ascontiguousarray(np.asarray(values, np.float32))
    tau = np.asarray(tau, np.float32)
    delta = np.ascontiguousarray(np.asarray(delta, np.float32))
    shared = {
        "Wq": np.ascontiguousarray(np.asarray(Wq, np.float32)),
        "Wk": np.ascontiguousarray(np.asarray(Wk, np.float32)),
        "Wv": np.ascontiguousarray(np.asarray(Wv, np.float32)),
        "Wo": np.ascontiguousarray(np.asarray(Wo, np.float32)),
        "bq": np.ascontiguousarray(np.asarray(bq, np.float32)),
        "bk": np.ascontiguousarray(np.asarray(bk, np.float32)),
        "bv": np.ascontiguousarray(np.asarray(bv, np.float32)),
        "bo": np.ascontiguousarray(np.asarray(bo, np.float32)),
    }

    in_maps = []
    for c in range(NCORES):
        b, hf = divmod(c, 2)
        in_maps.append({
            "qTin": np.ascontiguousarray(
                queries[b, hf * LC:(hf + 1) * LC, :].T),
            "kTin": np.ascontiguousarray(keys[b].T),
            "vTin": np.ascontiguousarray(values[b].T),
            "tau": np.ascontiguousarray(tau[b:b + 1]),
            "delta": np.ascontiguousarray(delta[b]),
            **shared,
        })

    nc = _get_nc()
    res = run_bass_kernel_spmd(nc, in_maps, core_ids=list(range(NCORES)))

    out = np.empty((B, LFULL, D), np.float32)
    for c in range(NCORES):
        b, hf = divmod(c, 2)
        out[b, hf * LC:(hf + 1) * LC, :] = res.results[c]["out"]
    return out


# revision 20
# speedup vs baseline: 1.1846x; 1.1846x over previous
"""De-stationary attention (B=4, L=S=2048, D=512, H=8, dk=64) on 8 TRN2 cores.

Sharding: core c -> batch b = c//2, query-half = c%2 (1024 rows each).
Each core computes full attention for its (batch, q-half) over all 8 heads
using the whole K/V of that batch; outputs concatenate with no reduction.

Math (per batch):
  q = queries @ Wq + bq ; k = keys @ Wk + bk ; v = values @ Wv + bv
  scores = tau * (q . k) / 8 + delta[s]
  attn   = softmax_s(scores)            (no max-subtraction; |scores| <~ 10)
  out    = (attn @ v) @ Wo + bo

Device-side tricks:
  exp(tau*qk/8 + delta_s) = exp(tau/8 * qk) * w_s with w_s = exp(delta_s)
  folded into V: the AV matmul uses lhsT = [w*v | w] so row 64 of the
  (transposed) AV output accumulates the softmax denominator.
  Layouts are transposed end-to-end (host supplies X^T inputs) so no
  on-device transposes are needed; the final output is natural [q, d].
  The projection passes for k/q are emitted per head-pair (jo == hp) and
  interleaved with attention so ScalarE (the exp bottleneck) starts early.
"""

import os
from contextlib import ExitStack

import numpy as np

import concourse.bass as bass
import concourse.bacc as bacc
import concourse.mybir as mybir
import concourse.tile as tile
from concourse.bass_utils import run_bass_kernel_spmd

# Problem constants (hardcoded per the harness contract).
B, LFULL, S, D = 4, 2048, 2048, 512
H, DK = 8, 64
NCORES = 8
LC = B * LFULL // NCORES  # 1024 query rows per core
NQT = LC // 512           # q-tiles of 512
SC = S // 128             # 16 s-chunks
F32 = mybir.dt.float32
F32R = mybir.dt.float32r

# Matmul dtype knob: "f32r" (full-rate, ~tf32 precision) or "f32" (4x slower).
MM_DTYPE = os.environ.get("KERNEL_MM_DTYPE", "f32r")
MDT = F32R if MM_DTYPE == "f32r" else F32
AF = mybir.ActivationFunctionType
OP = mybir.AluOpType


def _mm(nc, out, lhsT, rhs, **kw):
    nc.tensor.matmul(out, lhsT, rhs, **kw)


def build_nc():
    nc = bacc.Bacc()

    qTin = nc.dram_tensor("qTin", [D, LC], MDT, kind="ExternalInput")
    kTin = nc.dram_tensor("kTin", [D, S], MDT, kind="ExternalInput")
    vTin = nc.dram_tensor("vTin", [D, S], MDT, kind="ExternalInput")
    Wq = nc.dram_tensor("Wq", [D, D], MDT, kind="ExternalInput")
    Wk = nc.dram_tensor("Wk", [D, D], MDT, kind="ExternalInput")
    Wv = nc.dram_tensor("Wv", [D, D], MDT, kind="ExternalInput")
    Wo = nc.dram_tensor("Wo", [D, D], MDT, kind="ExternalInput")
    bq = nc.dram_tensor("bq", [D], F32, kind="ExternalInput")
    bk = nc.dram_tensor("bk", [D], F32, kind="ExternalInput")
    bv = nc.dram_tensor("bv", [D], MDT, kind="ExternalInput")
    bo = nc.dram_tensor("bo", [D], MDT, kind="ExternalInput")
    tau = nc.dram_tensor("tau", [1], F32, kind="ExternalInput")
    delta = nc.dram_tensor("delta", [S], F32, kind="ExternalInput")
    out = nc.dram_tensor("out", [LC, D], F32, kind="ExternalOutput")

    kTin_r = kTin.rearrange("(j p) s -> p j s", p=128)
    qTin_r = qTin.rearrange("(j p) l -> p j l", p=128)
    vTin_r = vTin.rearrange("(j p) s -> p j s", p=128)

    with ExitStack() as ctx:
        tc = ctx.enter_context(tile.TileContext(nc))
        consts = ctx.enter_context(tc.tile_pool(name="consts", bufs=1))
        proj = ctx.enter_context(tc.tile_pool(name="proj", bufs=1))
        pin = ctx.enter_context(tc.tile_pool(name="pin", bufs=1))
        kqr = ctx.enter_context(tc.tile_pool(name="kqr", bufs=2))
        vsl = ctx.enter_context(tc.tile_pool(name="vsl", bufs=2))
        pp = ctx.enter_context(tc.tile_pool(name="pp", bufs=3))
        onp = ctx.enter_context(tc.tile_pool(name="onp", bufs=15))
        rcp = ctx.enter_context(tc.tile_pool(name="rcp", bufs=2))
        dtp = ctx.enter_context(tc.tile_pool(name="dtp", bufs=4))
        epp = ctx.enter_context(tc.tile_pool(name="epp", bufs=2))
        fsp = ctx.enter_context(tc.tile_pool(name="fsp", bufs=1))
        drp = ctx.enter_context(tc.tile_pool(name="drp", bufs=4, space="DRAM"))
        qkp = ctx.enter_context(tc.tile_pool(name="qkp", bufs=3, space="PSUM"))
        avp = ctx.enter_context(tc.tile_pool(name="avp", bufs=2, space="PSUM"))

        # --- small constants -------------------------------------------------
        bq_sb = consts.tile([128, 4], F32)
        nc.sync.dma_start(out=bq_sb, in_=bq.rearrange("(j p) -> p j", p=128))
        bk_sb = consts.tile([128, 4], F32)
        nc.sync.dma_start(out=bk_sb, in_=bk.rearrange("(j p) -> p j", p=128))
        bv_row = consts.tile([1, D], MDT)
        nc.sync.dma_start(out=bv_row, in_=bv.rearrange("(a n) -> a n", a=1))
        bo_row = consts.tile([1, D], MDT)
        nc.sync.dma_start(out=bo_row, in_=bo.rearrange("(a n) -> a n", a=1))
        tau_bc0 = consts.tile([128, 1], F32)
        nc.sync.dma_start(
            out=tau_bc0,
            in_=tau.rearrange("(a b) -> a b", a=1).to_broadcast([128, 1]))
        tau_bc = consts.tile([128, 1], F32)
        nc.vector.tensor_scalar(out=tau_bc, in0=tau_bc0, scalar1=0.125,
                                scalar2=None, op0=OP.mult)  # tau/sqrt(dk)
        delta_sb = consts.tile([128, SC], F32)
        nc.sync.dma_start(out=delta_sb, in_=delta.rearrange("(j p) -> p j", p=128))
        ones_f32 = consts.tile([1, 128], F32)
        nc.vector.memset(ones_f32, 1.0)
        ones_sb = consts.tile([1, 128], MDT)
        nc.vector.tensor_copy(out=ones_sb, in_=ones_f32)
        w_sb = consts.tile([128, SC], F32)  # w[s] = exp(delta[s])
        nc.scalar.activation(w_sb, delta_sb, AF.Exp)

        # big inputs, consumption-ordered. vTin/Wv ride the scalar-engine
        # HWDGE queue so they stream in parallel with the k/q input DMAs.
        Wv_sb = consts.tile([128, 4, D], MDT)
        nc.scalar.dma_start(out=Wv_sb, in_=Wv.rearrange("(j p) n -> p j n", p=128))
        Wk_sb = consts.tile([128, 4, D], MDT)
        nc.sync.dma_start(out=Wk_sb, in_=Wk.rearrange("(j p) n -> p j n", p=128))
        kTin_sb = pin.tile([128, 4, S], MDT)
        nc.sync.dma_start(out=kTin_sb[:, :, 0:512], in_=kTin_r[:, :, 0:512])
        Wq_sb = consts.tile([128, 4, D], MDT)
        nc.sync.dma_start(out=Wq_sb, in_=Wq.rearrange("(j p) n -> p j n", p=128))
        qTin_sb = pin.tile([128, 4, LC], MDT)
        nc.sync.dma_start(out=qTin_sb[:, :, 0:512], in_=qTin_r[:, :, 0:512])
        for st in range(1, 4):
            nc.sync.dma_start(out=kTin_sb[:, :, st * 512:(st + 1) * 512],
                              in_=kTin_r[:, :, st * 512:(st + 1) * 512])
        nc.sync.dma_start(out=qTin_sb[:, :, 512:1024], in_=qTin_r[:, :, 512:1024])
        Wo_sb = consts.tile([64, H, D], MDT)  # Wo rows of head h at parts 0..63
        nc.sync.dma_start(out=Wo_sb, in_=Wo.rearrange("(h d) n -> d h n", d=64))

        # persistent across all phases: weighted values [w*v | w]
        vw_sb = proj.tile([128, SC, H, 65], MDT)

        outTn = {}

        def emit_vproj(st):
            vsl_t = vsl.tile([128, 4, 128], MDT, name=f"vin_{st}", tag="vin")
            nc.scalar.dma_start(out=vsl_t,
                                in_=vTin_r[:, :, st * 128:(st + 1) * 128])
            ps = qkp.tile([128, 512], F32, name=f"psv_{st}", tag="qk",
                          padded_shape=[128, 1024])
            for ji in range(4):
                _mm(nc, ps, vsl_t[:, ji, :], Wv_sb[:, ji, :],
                    start=(ji == 0), stop=False)
            _mm(nc, ps, ones_sb, bv_row, start=False, stop=True)
            nc.vector.tensor_scalar(
                out=vw_sb[:, st, :, 0:64],
                in0=ps.rearrange("p (h d) -> p h d", h=H),
                scalar1=w_sb[:, st:st + 1], scalar2=None, op0=OP.mult)
            nc.vector.tensor_copy(
                out=vw_sb[:, st, :, 64:65],
                in_=w_sb[:, st:st + 1].to_broadcast([128, H, 1]))

        for st in range(SC):
            emit_vproj(st)

        for hp in range(H // 2):
            h0, h1 = 2 * hp, 2 * hp + 1
            # k/q projection pass for this head pair (jo == hp)
            kT_sb = kqr.tile([128, S], MDT, name=f"kT_{hp}", tag="kT")
            for st in range(4):
                ps = qkp.tile([128, 512], F32, name=f"psk_{hp}_{st}", tag="qk",
                              padded_shape=[128, 1024])
                for ji in range(4):
                    _mm(nc, ps, Wk_sb[:, ji, hp * 128:(hp + 1) * 128],
                        kTin_sb[:, ji, st * 512:(st + 1) * 512],
                        start=(ji == 0), stop=(ji == 3))
                nc.vector.tensor_scalar(
                    out=kT_sb[:, st * 512:(st + 1) * 512], in0=ps,
                    scalar1=bk_sb[:, hp:hp + 1], scalar2=None, op0=OP.add)
            qT_sb = kqr.tile([128, LC], MDT, name=f"qT_{hp}", tag="qT")
            for lt in range(NQT):
                ps = qkp.tile([128, 512], F32, name=f"psq_{hp}_{lt}", tag="qk",
                              padded_shape=[128, 1024])
                for ji in range(4):
                    _mm(nc, ps, Wq_sb[:, ji, hp * 128:(hp + 1) * 128],
                        qTin_sb[:, ji, lt * 512:(lt + 1) * 512],
                        start=(ji == 0), stop=(ji == 3))
                nc.vector.tensor_scalar(
                    out=qT_sb[:, lt * 512:(lt + 1) * 512], in0=ps,
                    scalar1=bq_sb[:, hp:hp + 1], scalar2=None, op0=OP.add)

            for qt in range(NQT):
                av = [avp.tile([128, 512], F32, name=f"av_{qt}_{hp}_{j}",
                               tag="avf") for j in range(2)]
                for scp in range(SC // 2):
                    qk0 = qkp.tile([128, 1024], F32,
                                   name=f"qk0_{qt}_{hp}_{scp}", tag="qk")
                    qk1 = qkp.tile([128, 1024], F32,
                                   name=f"qk1_{qt}_{hp}_{scp}", tag="qk")
                    for k2 in range(2):
                        sc = 2 * scp + k2
                        # heads of the pair live on partition halves of the
                        # kT/qT pass tiles -> concurrent row-tiled matmuls
                        _mm(nc, qk0[:, k2 * 512:(k2 + 1) * 512],
                            kT_sb[0:64, sc * 128:(sc + 1) * 128],
                            qT_sb[0:64, qt * 512:(qt + 1) * 512],
                            start=True, stop=True)
                        _mm(nc, qk1[:, k2 * 512:(k2 + 1) * 512],
                            kT_sb[64:128, sc * 128:(sc + 1) * 128],
                            qT_sb[64:128, qt * 512:(qt + 1) * 512],
                            start=True, stop=True)
                    p0 = pp.tile([128, 1024], MDT,
                                 name=f"p0_{qt}_{hp}_{scp}", tag="p")
                    p1 = pp.tile([128, 1024], MDT,
                                 name=f"p1_{qt}_{hp}_{scp}", tag="p")
                    nc.scalar.activation(p0, qk0, AF.Exp, scale=tau_bc)
                    nc.scalar.activation(p1, qk1, AF.Exp, scale=tau_bc)
                    for k2 in range(2):
                        sc = 2 * scp + k2
                        _mm(nc, av[0][0:65, :], vw_sb[:, sc, h0, :],
                            p0[:, k2 * 512:(k2 + 1) * 512],
                            start=(sc == 0), stop=(sc == SC - 1))
                        _mm(nc, av[1][0:65, :], vw_sb[:, sc, h1, :],
                            p1[:, k2 * 512:(k2 + 1) * 512],
                            start=(sc == 0), stop=(sc == SC - 1))
                for i2, h in ((0, h0), (1, h1)):
                    # copy [numer | denom] out of PSUM at once (frees the AV
                    # bank), then: denom row -> DRAM -> [128,4] spread ->
                    # reciprocal -> DRAM -> partition-broadcast [128,512]
                    rt = rcp.tile([65, 512], F32, name=f"rt_{qt}_{h}", tag="rt")
                    nc.vector.tensor_copy(out=rt, in_=av[i2][0:65, :])
                    d1 = drp.tile([512], F32, name=f"d1_{qt}_{h}", tag="d1")
                    nc.sync.dma_start(out=d1.rearrange("(a n) -> a n", a=1),
                                      in_=rt[64:65, :])
                    denT = dtp.tile([128, 4], F32, name=f"dT_{qt}_{h}", tag="dT")
                    nc.sync.dma_start(out=denT,
                                      in_=d1.rearrange("(j p) -> p j", p=128))
                    recipT = dtp.tile([128, 4], F32, name=f"rT_{qt}_{h}",
                                      tag="rT")
                    nc.vector.reciprocal(recipT, denT)
                    d2 = drp.tile([512], F32, name=f"d2_{qt}_{h}", tag="d2")
                    nc.sync.dma_start(out=d2.rearrange("(j p) -> p j", p=128),
                                      in_=recipT)
                    rb = epp.tile([128, 512], F32, name=f"rb_{qt}_{h}", tag="rb")
                    nc.sync.dma_start(
                        out=rb,
                        in_=d2.rearrange("(a n) -> a n", a=1)
                        .to_broadcast([128, 512]))
                    ot = onp.tile([64, 512], MDT, name=f"ot_{qt}_{h}", tag="ot")
                    nc.vector.tensor_mul(ot, rt[0:64, :], rb[0:64, :])
                    outTn[(qt, h)] = ot

                if hp == H // 2 - 1:
                    # all heads of this q-tile done: output projection
                    for i in range(4):
                        fps = avp.tile([128, 512], F32, name=f"fps_{qt}_{i}",
                                       tag="avf")
                        for h in range(H):
                            _mm(nc, fps, outTn[(qt, h)][:, i * 128:(i + 1) * 128],
                                Wo_sb[:, h, :], start=(h == 0), stop=False)
                        _mm(nc, fps, ones_sb, bo_row, start=False, stop=True)
                        fsb = fsp.tile([128, 512], F32, name=f"fsb_{qt}_{i}",
                                       tag="fsb")
                        nc.vector.tensor_copy(out=fsb, in_=fps)
                        r0 = qt * 512 + i * 128
                        nc.sync.dma_start(out=out[r0:r0 + 128, :], in_=fsb)

    return nc


_NC_CACHE = None


def _get_nc():
    global _NC_CACHE
    if _NC_CACHE is None:
        _NC_CACHE = build_nc()
        _NC_CACHE.finalize()
    return _NC_CACHE


def kernel(queries, keys, values, tau, delta, Wq, bq, Wk, bk, Wv, bv, Wo, bo,
           **_unused):
    queries = np.ascontiguousarray(np.asarray(queries, np.float32))
    keys = np.ascontiguousarray(np.asarray(keys, np.float32))
    values = np.ascontiguousarray(np.asarray(values, np.float32))
    tau = np.asarray(tau, np.float32)
    delta = np.ascontiguousarray(np.asarray(delta, np.float32))
    shared = {
        "Wq": np.ascontiguousarray(np.asarray(Wq, np.float32)),
        "Wk": np.ascontiguousarray(np.asarray(Wk, np.float32)),
        "Wv": np.ascontiguousarray(np.asarray(Wv, np.float32)),
        "Wo": np.ascontiguousarray(np.asarray(Wo, np.float32)),
        "bq": np.ascontiguousarray(np.asarray(bq, np.float32)),
        "bk": np.ascontiguousarray(np.asarray(bk, np.float32)),
        "bv": np.ascontiguousarray(np.asarray(bv, np.float32)),
        "bo": np.ascontiguousarray(np.asarray(bo, np.float32)),
    }

    in_maps = []
    for c in range(NCORES):
        b, hf = divmod(c, 2)
        in_maps.append({
            "qTin": np.ascontiguousarray(
                queries[b, hf * LC:(hf + 1) * LC, :].T),
            "kTin": np.ascontiguousarray(keys[b].T),
            "vTin": np.ascontiguousarray(values[b].T),
            "tau": np.ascontiguousarray(tau[b:b + 1]),
            "delta": np.ascontiguousarray(delta[b]),
            **shared,
        })

    nc = _get_nc()
    res = run_bass_kernel_spmd(nc, in_maps, core_ids=list(range(NCORES)))

    out = np.empty((B, LFULL, D), np.float32)
    for c in range(NCORES):
        b, hf = divmod(c, 2)
        out[b, hf * LC:(hf + 1) * LC, :] = res.results[c]["out"]
    return out
